# revision 1
# baseline (speedup 1.0000x reference)
"""GroupMixAttention Trainium2 kernel (8-core SPMD, batch-parallel).

Problem: x[16,256,32,32]; per group g (4 groups of 64 ch):
  Q/K/V = wq/wk/wv[g] @ xg   (xg = [64, 1024])
  scores = (Q^T K)/8 ; attn = softmax(scores, -1) ; out = V @ attn^T
then y = wo @ concat(out).

Sharding: data-parallel over batch, 2 batches per core, no collectives.

Layout strategy per (batch, group-pair):
  - x2 [128, 1024] holds two groups' channels (natural slicing of x).
  - Q2/K2 [128, 1024] computed with row+col tiled matmuls (two 64-row
    groups concurrently on the PE array).
  - scoresT[m, n] = K^T Q computed chunk-wise (m in 8 chunks of 128) with
    the two groups packed in PE row-halves; exp on the scalar engine
    (scale=1/8 folded in; softmax max-subtraction skipped — scores are
    O(5) so fp32 exp is safe).
  - V^T chunks [128(m), 64(d)] computed directly (lhsT = x chunks) with a
    ones column appended -> PV matmul lhsT [128, 65]: row 64 of the
    accumulated output is the softmax denominator.
  - E and V^T are cast to fp16 so the K=128 PV matmul is single-pass
    (fp32 at K=128 is two-pass/half-rate on the PE array).
  - normalize (deferred to batch end, off the PE critical path): denom rows
    staged at 32-aligned partitions -> one batched DVE reciprocal -> K=1
    ones-matmul broadcast into PSUM -> DVE multiply.
  - out_proj: wo^T chunks as lhsT over the stacked normalized heads.
  - mc loop is software-pipelined (scores issued 2 steps ahead of exp+PV)
    so the PE never stalls on the scalar engine's exp.
"""

import os
import sys

import numpy as np

for _p in ("/opt/trn_rl_repo", "/root/.axon_site/_ro/trn_rl_repo"):
    if os.path.isdir(_p) and _p not in sys.path:
        sys.path.insert(0, _p)

import concourse.bass as bass
import concourse.mybir as mybir
import concourse.tile as tile
from concourse import bacc
from concourse.bass_utils import run_bass_kernel_spmd

F32 = mybir.dt.float32
BF16 = mybir.dt.float16
EXP = mybir.ActivationFunctionType.Exp
N_CORES = 8
B_PER_CORE = 2  # 16 batches / 8 cores
NT = 1024  # H*W
GD = 64    # group dim
ts = bass.ts


def _build_program():
    nc = bacc.Bacc("TRN2", target_bir_lowering=False, debug=False,
                   num_devices=N_CORES)
    xs = nc.dram_tensor("xs", [B_PER_CORE, 2, 128, NT], F32,
                        kind="ExternalInput").ap()
    wqT = nc.dram_tensor("wqT", [2, 128, GD], F32, kind="ExternalInput").ap()
    wkT = nc.dram_tensor("wkT", [2, 128, GD], F32, kind="ExternalInput").ap()
    wvT = nc.dram_tensor("wvT", [2, 128, GD], F32, kind="ExternalInput").ap()
    woT = nc.dram_tensor("woT", [2, 128, 256], F32, kind="ExternalInput").ap()
    y = nc.dram_tensor("y", [B_PER_CORE, 256, NT], F32,
                       kind="ExternalOutput").ap()

    with tile.TileContext(nc) as tc:
        from contextlib import ExitStack
        with ExitStack() as ctx:
            const = ctx.enter_context(tc.tile_pool(name="const", bufs=1))
            xpool = ctx.enter_context(tc.tile_pool(name="xp", bufs=2))
            qk = ctx.enter_context(tc.tile_pool(name="qk", bufs=2))
            vtp = ctx.enter_context(tc.tile_pool(name="vt", bufs=2))
            ep = ctx.enter_context(tc.tile_pool(name="ep", bufs=3))
            sm = ctx.enter_context(tc.tile_pool(name="sm", bufs=2))
            onp = ctx.enter_context(tc.tile_pool(name="on", bufs=2))
            yp = ctx.enter_context(tc.tile_pool(name="yp", bufs=2))
            psS = ctx.enter_context(
                tc.tile_pool(name="psS", bufs=3, space="PSUM"))
            psAcc = ctx.enter_context(
                tc.tile_pool(name="psAcc", bufs=1, space="PSUM"))

            # Load weights once.
            w_sb = {}
            for name, dram in (("wq", wqT), ("wk", wkT), ("wv", wvT)):
                for p in range(2):
                    t = const.tile([128, GD], F32, tag=f"{name}{p}", name=f"{name}{p}")
                    nc.sync.dma_start(t[:], dram[p])
                    w_sb[name, p] = t
            ones128 = const.tile([128, 128], F32, tag="ones128",
                                 name="ones128")
            nc.gpsimd.memset(ones128[:], 1.0)
            wo_sb = []
            for k in range(2):
                t = const.tile([128, 256], F32, tag=f"wo{k}", name=f"wo{k}")
                nc.sync.dma_start(t[:], woT[k])
                wo_sb.append(t)

            for b in range(B_PER_CORE):
                outN = [onp.tile([128, NT], F32, tag=f"outN{p}", name=f"outN{p}")
                        for p in range(2)]
                norm_jobs = []
                for p in range(2):
                    x2 = xpool.tile([128, NT], F32, tag="x2")
                    nc.sync.dma_start(x2[:], xs[b, p])
                    den4 = sm.tile([128, 512], F32, tag="den4")

                    # K2 / Q2 projections, both groups packed on the array.
                    K2 = qk.tile([128, NT], F32, tag="K2")
                    Q2 = qk.tile([128, NT], F32, tag="Q2")
                    for wname, dst in (("wk", K2), ("wq", Q2)):
                        wt = w_sb[wname, p]
                        for nh in range(2):
                            s = ts(nh, 512)
                            ps = psS.tile([128, 512], F32, tag=f"pss{nh}",
                                          name=f"qkv{nh}")
                            nc.tensor.matmul(
                                ps[0:64, :], wt[0:64, :], x2[0:64, s],
                                start=True, stop=True, tile_position=(0, 0))
                            nc.tensor.matmul(
                                ps[64:128, :], wt[64:128, :], x2[64:128, s],
                                start=True, stop=True, tile_position=(64, 64))
                            nc.vector.tensor_copy(dst[:, s], ps[:])

                    # V^T chunks with ones column (denominator trick).
                    vts = [vtp.tile([128, 8 * (GD + 1)], BF16, tag=f"vt{g}", name=f"vt{g}")
                           for g in range(2)]
                    for g in range(2):
                        nc.vector.memset(vts[g][:], 1.0)
                    wv = w_sb["wv", p]
                    for mc in range(8):
                        pvA = psS.tile([128, GD], F32, tag="pss0")
                        pvB = psS.tile([128, GD], F32, tag="pss1")
                        nc.tensor.matmul(
                            pvA[:], x2[0:64, ts(mc, 128)], wv[0:64, :],
                            start=True, stop=True, tile_position=(0, 0))
                        nc.tensor.matmul(
                            pvB[:], x2[64:128, ts(mc, 128)], wv[64:128, :],
                            start=True, stop=True, tile_position=(64, 0))
                        c0 = 65 * mc
                        nc.vector.tensor_copy(vts[0][:, c0:c0 + GD], pvA[:])
                        nc.vector.tensor_copy(vts[1][:, c0:c0 + GD], pvB[:])

                    # Attention, n in two 512-halves to bound PSUM usage.
                    for nh in range(2):
                        ns = ts(nh, 512)
                        psO = [psAcc.tile([GD + 1, 512], F32, tag=f"psO{g}", name=f"psO{g}")
                               for g in range(2)]
                        sc = {}
                        for step in range(10):
                            if step < 8:
                                msl = ts(step, 128)
                                pss = [psS.tile([128, 512], F32,
                                                tag=f"pss{g}",
                                                name=f"pss{g}_{step}")
                                       for g in range(2)]
                                nc.tensor.matmul(
                                    pss[0][:], K2[0:64, msl], Q2[0:64, ns],
                                    start=True, stop=True,
                                    tile_position=(0, 0))
                                nc.tensor.matmul(
                                    pss[1][:], K2[64:128, msl],
                                    Q2[64:128, ns],
                                    start=True, stop=True,
                                    tile_position=(64, 0))
                                sc[step] = pss
                            if step >= 2:
                                mc = step - 2
                                for g in range(2):
                                    E = ep.tile([128, 512], BF16, tag=f"E{g}",
                                                name=f"E{g}_{mc}")
                                    nc.scalar.activation(
                                        E[:], sc[mc][g][:], EXP, scale=0.125)
                                    c0 = 65 * mc
                                    nc.tensor.matmul(
                                        psO[g][:], vts[g][:, c0:c0 + GD + 1],
                                        E[:], start=(mc == 0), stop=(mc == 7))
                        # stage numerators + denominators; normalize later
                        # (keeps the slow single-partition reciprocal off
                        # the PE critical path and frees psO banks early)
                        for g in range(2):
                            nc.vector.tensor_copy(
                                outN[p][GD * g:GD * (g + 1), ns],
                                psO[g][0:GD, :])
                            r = 32 * (2 * nh + g)
                            nc.vector.tensor_copy(
                                den4[r:r + 1, :], psO[g][GD:GD + 1, :])

                    # batched reciprocal now (DVE, overlaps next pair);
                    # broadcast+multiply deferred to batch end
                    rec4 = sm.tile([128, 512], F32, tag="rec4",
                                   name=f"rec4_{p}")
                    nc.vector.reciprocal(rec4[:], den4[:])
                    norm_jobs.append((p, rec4))

                for p, rec4 in norm_jobs:
                    for nh in range(2):
                        ns = ts(nh, 512)
                        psR = psS.tile([128, 512], F32, tag=f"pss{nh}",
                                       name=f"psR{nh}_{p}")
                        for g in range(2):
                            r = 32 * (2 * nh + g)
                            nc.tensor.matmul(
                                psR[GD * g:GD * (g + 1), :],
                                ones128[r:r + 1, 0:GD], rec4[r:r + 1, :],
                                start=True, stop=True,
                                tile_position=(r, GD * g))
                            rows = outN[p][GD * g:GD * (g + 1), ns]
                            nc.vector.tensor_mul(
                                rows, rows, psR[GD * g:GD * (g + 1), :])

                # out_proj: y[b] = woT.T @ outN (contraction over C=256)
                for ec in range(2):
                    yt = yp.tile([128, NT], F32, tag="yt")
                    for nh in range(2):
                        s = ts(nh, 512)
                        psY = psS.tile([128, 512], F32, tag=f"pss{nh}",
                                       name=f"psY{nh}")
                        for kc in range(2):
                            nc.tensor.matmul(
                                psY[:], wo_sb[kc][:, ts(ec, 128)],
                                outN[kc][:, s],
                                start=(kc == 0), stop=(kc == 1))
                        nc.vector.tensor_copy(yt[:, s], psY[:])
                    nc.sync.dma_start(y[b][ts(ec, 128), :], yt[:])

    nc.finalize()
    return nc


_NC_CACHE = None


def _get_nc():
    global _NC_CACHE
    if _NC_CACHE is None:
        _NC_CACHE = _build_program()
    return _NC_CACHE


def _prep_inputs(x, wq, wk, wv, wo):
    B = x.shape[0]
    xr = np.ascontiguousarray(x.reshape(B, 2, 128, NT), dtype=np.float32)
    # [G, d, c] -> [G, c, d] -> [pair, 128, d]
    wqT = np.ascontiguousarray(
        wq.transpose(0, 2, 1).reshape(2, 128, GD), dtype=np.float32)
    wkT = np.ascontiguousarray(
        wk.transpose(0, 2, 1).reshape(2, 128, GD), dtype=np.float32)
    wvT = np.ascontiguousarray(
        wv.transpose(0, 2, 1).reshape(2, 128, GD), dtype=np.float32)
    woT = np.ascontiguousarray(wo.T.reshape(2, 128, 256), dtype=np.float32)
    return xr, wqT, wkT, wvT, woT


def run(x, wq, wk, wv, wo, trace=False, **trace_kwargs):
    x = np.asarray(x, dtype=np.float32)
    B, C, H, W = x.shape
    xr, wqT, wkT, wvT, woT = _prep_inputs(
        x, np.asarray(wq, np.float32), np.asarray(wk, np.float32),
        np.asarray(wv, np.float32), np.asarray(wo, np.float32))
    in_maps = []
    for c in range(N_CORES):
        in_maps.append({
            "xs": xr[c * B_PER_CORE:(c + 1) * B_PER_CORE],
            "wqT": wqT, "wkT": wkT, "wvT": wvT, "woT": woT,
        })
    res = run_bass_kernel_spmd(_get_nc(), in_maps, list(range(N_CORES)),
                               trace=trace, **trace_kwargs)
    outs = [res.results[c]["y"] for c in range(N_CORES)]
    yfull = np.concatenate(outs, axis=0).reshape(B, C, H, W)
    return yfull.astype(np.float32), res


def kernel(x, wq, wk, wv, wo):
    out, _ = run(x, wq, wk, wv, wo, trace=False)
    return out



# revision 15
# speedup vs baseline: 1.4875x; 1.4875x over previous
"""GroupMixAttention Trainium2 kernel (8-core SPMD, batch-parallel).

Problem: x[16,256,32,32]; per group g (4 groups of 64 ch):
  Q/K/V = wq/wk/wv[g] @ xg   (xg = [64, 1024])
  scores = (Q^T K)/8 ; attn = softmax(scores, -1) ; out = V @ attn^T
then y = wo @ concat(out).

Sharding: data-parallel over batch, 2 batches per core, no collectives.

V3 design (vs fp32 baseline):
  - ALL matmul operands fp16 (4x fewer PE cycles/row than fp32; host
    pre-casts x and builds block-diagonal per-pair weights so each
    projection is one M=128/K=128 instruction per 512 cols).
  - scores S^T[m,n] = K^T Q per group: 2 instrs/mc-chunk (ap=512).
  - exp: split across engines. Act engine computes 16*exp(s/8) (bias
    ln16); a ~31% fraction of (g,mc) chunks instead use a 1-instruction
    Schraudolph exp2 on DVE: i16 = s*(log2e/8*1024) + (19*1024+c),
    bit-viewed as fp16 == 2^(s/8*log2e + 4) = 16*exp(s/8) (+-2% sawtooth;
    softmax scale-invariance cancels the 2^4; c=-50 zeroes the relative
    bias between exact and approx chunks).  (Pool/gpsimd cannot read
    PSUM on trn2, so only Act+DVE drain PSUM.)
  - PV computed transposed: O^T[n,d] = sum_m E[m,n]*V^T[m,d] with E as
    the stationary operand -> M=128 (vs 65 when V^T is stationary),
    halving PV cost.  PSUM accumulation groups must be exclusive per
    2KB bank and must not cross banks, so PV runs ns-outer/mc-inner
    (one 64-float region per group, opened and closed within a step)
    and is software-pipelined one unit late against the next unit's
    scores.  A paired ap=1 matmul with a ones vector (same stationary E,
    no reload) accumulates the softmax denominator into a separate
    psDen bank.
  - normalize: DVE drains PSUM once; Pool (SBUF-only) does the
    per-partition reciprocal-scale multiplies into fp16.
  - PE transpose (identity matmul) puts O back in [d,n] layout for the
    out_proj; epilogues are emitted one phase late so the PE never waits
    on the normalize chain.
  - out_proj: wo^T chunks fp16 over the two pair tiles.
"""

import os
import sys

import numpy as np

for _p in ("/opt/trn_rl_repo", "/root/.axon_site/_ro/trn_rl_repo"):
    if os.path.isdir(_p) and _p not in sys.path:
        sys.path.insert(0, _p)

import concourse.bass as bass
import concourse.mybir as mybir
import concourse.tile as tile
from concourse import bacc
from concourse import masks
from concourse.bass_utils import run_bass_kernel_spmd

F32 = mybir.dt.float32
F16 = mybir.dt.float16
I16 = mybir.dt.int16
EXP = mybir.ActivationFunctionType.Exp
MULT = mybir.AluOpType.mult
ADD = mybir.AluOpType.add
AMAX = mybir.AluOpType.max
N_CORES = 8
B_PER_CORE = 2  # 16 batches / 8 cores
NT = 1024  # H*W
GD = 64    # group dim
ts = bass.ts

# Q2h is pre-scaled by SCALE_Q so psS arrives as s*(log2e/8*1024); the
# Schraudolph path is then a single clamped add (max, add) and the act
# path rescales inside the activation.  E' = exp(s/8) * 2^-3 keeps fp16
# finite for raw |s|/8 up to ~12 (observed ~10.5).
SCALE_Q = (1.4426950408889634 / 8.0) * 1024.0
ACT_SCALE = 0.125 / SCALE_Q
ACT_BIAS = -3.0 * 0.6931471805599453    # ln(2^-3)
SCH_BI = 12.0 * 1024.0 - 50.0           # (15-3)*1024 + c, c=-50 bias-match

# exp engine per (g, mc): spread DVE offloads so Act never falls behind.
EXP_ENG = {
    (0, 0): "act", (0, 1): "act", (0, 2): "dve", (0, 3): "act",
    (0, 4): "act", (0, 5): "dve", (0, 6): "act", (0, 7): "act",
    (1, 0): "dve", (1, 1): "act", (1, 2): "act", (1, 3): "dve",
    (1, 4): "act", (1, 5): "act", (1, 6): "dve", (1, 7): "act",
}


def _build_program():
    nc = bacc.Bacc("TRN2", target_bir_lowering=False, debug=False,
                   num_devices=N_CORES)
    xh = nc.dram_tensor("xh", [B_PER_CORE, 2, 128, NT], F16,
                        kind="ExternalInput").ap()
    wqbd = nc.dram_tensor("wqbd", [2, 128, 128], F16, kind="ExternalInput").ap()
    wkbd = nc.dram_tensor("wkbd", [2, 128, 128], F16, kind="ExternalInput").ap()
    wvbd = nc.dram_tensor("wvbd", [2, 128, 128], F16, kind="ExternalInput").ap()
    woT = nc.dram_tensor("woT", [2, 128, 256], F16, kind="ExternalInput").ap()
    y = nc.dram_tensor("y", [B_PER_CORE, 256, NT], F32,
                       kind="ExternalOutput").ap()

    with tile.TileContext(nc) as tc:
        from contextlib import ExitStack
        with ExitStack() as ctx:
            const = ctx.enter_context(tc.tile_pool(name="const", bufs=1))
            xp = ctx.enter_context(tc.tile_pool(name="xp", bufs=2))
            qk = ctx.enter_context(tc.tile_pool(name="qk", bufs=2))
            vt = ctx.enter_context(tc.tile_pool(name="vt", bufs=2))
            ep = ctx.enter_context(tc.tile_pool(name="ep", bufs=18))
            otp = ctx.enter_context(tc.tile_pool(name="otp", bufs=3))
            orp = ctx.enter_context(tc.tile_pool(name="orp", bufs=2))
            dnp = ctx.enter_context(tc.tile_pool(name="dnp", bufs=2))
            onp = ctx.enter_context(tc.tile_pool(name="onp", bufs=2))
            yp = ctx.enter_context(tc.tile_pool(name="yp", bufs=2))
            psA = ctx.enter_context(
                tc.tile_pool(name="psA", bufs=3, space="PSUM"))
            psB = ctx.enter_context(
                tc.tile_pool(name="psB", bufs=2, space="PSUM"))

            # Load weights once.
            w_sb = {}
            for name, dram in (("wq", wqbd), ("wk", wkbd), ("wv", wvbd)):
                for p in range(2):
                    t = const.tile([128, 128], F16, tag=f"{name}{p}",
                                   name=f"{name}{p}")
                    nc.sync.dma_start(t[:], dram[p])
                    w_sb[name, p] = t
            wo_sb = []
            for kc in range(2):
                t = const.tile([128, 256], F16, tag=f"wo{kc}", name=f"wo{kc}")
                nc.sync.dma_start(t[:], woT[kc])
                wo_sb.append(t)
            ident = const.tile([128, 128], F16, tag="ident", name="ident")
            masks.make_identity(nc, ident[:])
            actb = const.tile([128, 1], F32, tag="actb", name="actb")
            nc.gpsimd.memset(actb[:], ACT_BIAS)
            ones1 = const.tile([128, 1], F16, tag="ones1", name="ones1")
            nc.gpsimd.memset(ones1[:], 1.0)

            x2h = {}       # pair -> [128, 1024] f16 input tile
            qkh = {}       # (pair, "wq"/"wk") -> [128, 1024] f16
            vtt = {}       # pair -> [128, 8, 2, 65] f16 V^T (+ones) tile
            Es = {}        # unit -> list of 8 E tiles (int16, fp16 bits)
            psOD = {}      # unit -> (psO [128,8,64] f32, psDen [128,8] f32)
            oT = {}        # (b, p) -> {g: [128, 512] f16 normalized O^T}
            outN = {}      # (b, p) -> [128, 1024] f16
            deferred = []  # post-phase work queue (epilogues, out_proj)

            def dma_x(b, p):
                t = xp.tile([128, NT], F16, tag="x2h", name=f"x2h_{b}{p}")
                nc.sync.dma_start(t[:], xh[b, p])
                x2h[b, p] = t

            def pair_setup(b, p):
                xt = x2h[b, p]
                # Q/K projections: block-diag weights, one instr per 512.
                for wname in ("wq", "wk"):
                    ps = psA.tile([128, NT], F32, tag="ps2b",
                                  name=f"ps{wname}_{b}{p}")
                    for nh in range(2):
                        nc.tensor.matmul(
                            ps[:, ts(nh, 512)], w_sb[wname, p][:],
                            xt[:, ts(nh, 512)], start=True, stop=True)
                    dst = qk.tile([128, NT], F16, tag=f"{wname}h",
                                  name=f"{wname}h_{b}{p}")
                    if wname == "wq":
                        nc.scalar.mul(dst[:], ps[:], SCALE_Q)
                    else:
                        nc.vector.tensor_copy(dst[:], ps[:])
                    qkh[b, p, wname] = dst

                # V^T chunks [m,128]=[V0|V1] + ones cols (PV rhs).
                VT = vt.tile([128, 8, 2, GD + 1], F16, tag="VT",
                             name=f"VT_{b}{p}")
                nc.vector.memset(VT[:, :, :, GD], 1.0)
                for h in range(2):
                    psV = psA.tile([128, 4, 2, GD], F32, tag="ps2b",
                                   name=f"psV{h}_{b}{p}")
                    for q4 in range(4):
                        mc = 4 * h + q4
                        nc.tensor.matmul(
                            psV[:, q4], xt[:, ts(mc, 128)],
                            w_sb["wv", p][:], start=True, stop=True)
                    nc.vector.tensor_copy(
                        VT[:, 4 * h:4 * h + 4, :, 0:GD], psV[:])
                vtt[b, p] = VT

            def sc_exp_step(u, mc):
                b, p, g = u
                gsl = slice(GD * g, GD * (g + 1))
                psS = psA.tile([128, NT], F32, tag="ps2b",
                               name=f"psS{g}{mc}_{b}{p}")
                for nh in range(2):
                    nc.tensor.matmul(
                        psS[:, ts(nh, 512)],
                        qkh[b, p, "wk"][gsl, ts(mc, 128)],
                        qkh[b, p, "wq"][gsl, ts(nh, 512)],
                        start=True, stop=True)
                E = ep.tile([128, NT], I16, tag="E",
                            name=f"E{g}{mc}_{b}{p}")
                if EXP_ENG[g, mc] == "act":
                    nc.scalar.activation(E[:].bitcast(F16), psS[:], EXP,
                                         scale=ACT_SCALE, bias=actb[:, 0:1])
                else:
                    nc.vector.tensor_scalar(
                        E[:], psS[:], -SCH_BI, SCH_BI, AMAX, ADD)
                Es[u].append(E)

            def pv_group(u, ns):
                """One PSUM accumulation group: O^T[n-chunk ns, d] plus its
                denominator, contracting over all 8 m-chunks."""
                b, p, g = u
                psO, psDen = psOD[u]
                VT = vtt[b, p]
                for mc in range(8):
                    Ec = Es[u][mc][:, ts(ns, 128)].bitcast(F16)
                    nc.tensor.matmul(
                        psO[:, ns], Ec, VT[:, mc, g, 0:GD],
                        start=(mc == 0), stop=(mc == 7),
                        skip_group_check=True)
                    nc.tensor.matmul(
                        psDen[:, ns:ns + 1], Ec, ones1[:],
                        start=(mc == 0), stop=(mc == 7),
                        skip_group_check=True)

            def normalize(u):
                b, p, g = u
                psO, psDen = psOD.pop(u)
                rden = dnp.tile([128, 8], F32, tag="rden",
                                name=f"rden{g}_{b}{p}")
                nc.vector.reciprocal(rden[:], psDen[:])
                oraw = orp.tile([128, 8, GD], F32, tag="oraw",
                                name=f"oraw{g}_{b}{p}")
                nc.vector.tensor_copy(oraw[:], psO[:])
                dst = otp.tile([128, 512], F16, tag="oT",
                               name=f"oT{g}_{b}{p}")
                for ns in range(8):
                    nc.gpsimd.tensor_scalar(
                        dst[:, ts(ns, GD)], oraw[:, ns],
                        rden[:, ns:ns + 1], None, MULT)
                oT.setdefault((b, p), {})[g] = dst
                Es[u] = None

            def epilogue(b, p):
                def run():
                    psT = psB.tile([128, NT], F16, tag="psO",
                                   name=f"psT_{b}{p}")
                    for g in range(2):
                        for j in range(8):
                            nc.tensor.transpose(
                                psT[GD * g:GD * (g + 1), ts(j, 128)],
                                oT[b, p][g][:, ts(j, GD)], ident[:])
                    o = onp.tile([128, NT], F16, tag="outN",
                                 name=f"outN_{b}{p}")
                    nc.vector.tensor_copy(o[:], psT[:])
                    outN[b, p] = o
                    if p == 1:
                        deferred.append(out_proj(b))
                return run

            def out_proj(b):
                def run():
                    for ec in range(2):
                        yt = yp.tile([128, NT], F32, tag="yt",
                                     name=f"yt{ec}_{b}")
                        for nh in range(2):
                            psY = psA.tile([128, 512], F32, tag="ps2b",
                                           name=f"psY{ec}{nh}_{b}")
                            for kc in range(2):
                                nc.tensor.matmul(
                                    psY[:], wo_sb[kc][:, ts(ec, 128)],
                                    outN[b, kc][:, ts(nh, 512)],
                                    start=(kc == 0), stop=(kc == 1))
                            nc.vector.tensor_copy(yt[:, ts(nh, 512)], psY[:])
                        nc.sync.dma_start(y[b][ts(ec, 128), :], yt[:])
                return run

            # --- emit: 2-stage software pipeline over the 8 (b,p,g) units
            units = [(b, p, g)
                     for b in range(B_PER_CORE) for p in range(2)
                     for g in range(2)]
            dma_x(0, 0)
            prev = None
            for u in units:
                b, p, g = u
                if g == 0:
                    pair_setup(b, p)
                    # prefetch next pair's input
                    nxt = [(bn, pn) for bn in range(B_PER_CORE)
                           for pn in range(2)
                           if (bn, pn) > (b, p)]
                    if nxt:
                        dma_x(*nxt[0])
                Es[u] = []
                psOD[u] = (
                    psB.tile([128, 8, GD], F32, tag="psO",
                             name=f"psO{g}_{b}{p}"),
                    psB.tile([128, 8], F32, tag="psO",
                             name=f"psDen{g}_{b}{p}"),
                )
                for mc in range(8):
                    sc_exp_step(u, mc)
                    if prev is not None:
                        pv_group(prev, mc)
                pending, deferred[:] = deferred[:], []
                if prev is not None:
                    normalize(prev)
                    if prev[2] == 1:
                        deferred.append(epilogue(prev[0], prev[1]))
                for w in pending:
                    w()
                prev = u
            # tail: drain the last unit
            for ns in range(8):
                pv_group(prev, ns)
            normalize(prev)
            deferred.append(epilogue(prev[0], prev[1]))
            while deferred:
                deferred.pop(0)()

    nc.finalize()
    return nc


_NC_CACHE = None


def _get_nc():
    global _NC_CACHE
    if _NC_CACHE is None:
        _NC_CACHE = _build_program()
    return _NC_CACHE


def _prep_inputs(x, wq, wk, wv, wo):
    B = x.shape[0]
    xr = np.ascontiguousarray(
        x.reshape(B, 2, 128, NT), dtype=np.float16)

    def bd(w):
        out = np.zeros((2, 128, 128), np.float16)
        for p in range(2):
            out[p, 0:GD, 0:GD] = w[2 * p].T
            out[p, GD:128, GD:128] = w[2 * p + 1].T
        return out

    woT = np.ascontiguousarray(wo.T.reshape(2, 128, 256), dtype=np.float16)
    return xr, bd(wq), bd(wk), bd(wv), woT


def run(x, wq, wk, wv, wo, trace=False, **trace_kwargs):
    x = np.asarray(x, dtype=np.float32)
    B, C, H, W = x.shape
    xr, wqbd, wkbd, wvbd, woT = _prep_inputs(
        x, np.asarray(wq, np.float32), np.asarray(wk, np.float32),
        np.asarray(wv, np.float32), np.asarray(wo, np.float32))
    in_maps = []
    for c in range(N_CORES):
        in_maps.append({
            "xh": xr[c * B_PER_CORE:(c + 1) * B_PER_CORE],
            "wqbd": wqbd, "wkbd": wkbd, "wvbd": wvbd, "woT": woT,
        })
    res = run_bass_kernel_spmd(_get_nc(), in_maps, list(range(N_CORES)),
                               trace=trace, **trace_kwargs)
    outs = [res.results[c]["y"] for c in range(N_CORES)]
    yfull = np.concatenate(outs, axis=0).reshape(B, C, H, W)
    return yfull.astype(np.float32), res


def kernel(x, wq, wk, wv, wo):
    out, _ = run(x, wq, wk, wv, wo, trace=False)
    return out
